# revision 40
# baseline (speedup 1.0000x reference)
"""Causal single-head attention on 8 trn2 NeuronCores, data-parallel over batch.

Reference computation (per batch element b):
  Q = x_b @ Wq.T + bq ; K = x_b @ Wk.T + bk ; V = x_b @ Wv.T + bv    (S=2048, D=A=1024)
  out_b = softmax(causal(Q K^T / 32)) V

Sharding: x is (S, B=8, D); core c handles batch element c. No collectives.

Per-core kernel design (v6: folded scores, fp8 DoubleRow G/V/PV, DMA-paced
phase interleave):
  - softmax is shift-invariant per row, so the q-side bias terms of
    (Q+bq)(K+bk)^T cancel; scores reduce to x M x^T + 1 v^T with M = Wq^T Wk
    folded on the HOST (weights-only folding) and v = x (Wk^T bq) computed on
    device (16 small PE matvec chains -> vsb[128,16], one column per key
    chunk). Removes both Q and K projections (-19% MACs vs the v2 baseline)
  - G pass in fp8 DoubleRow (M scaled x64 to clear the e4m3 subnormal floor,
    undone in the PSUM evacuation scale); the score matmul itself stays bf16:
    quantizing its operands too was measured at rel_err ~2e-2 = the gate
  - ST[k,q] = G_chunk.T @ xT_block; exp(SCALE*ps + vsb[kc]) on ScalarE (the
    k-side bias rides the per-partition activation bias slot; no row-max
    subtraction, |args| < ~2.5)
  - V projection and P@V run in fp8e4m3 with perf_mode=DoubleRow (2 fp8
    contraction planes per PE cell, moving free dim 512 so the doubled
    LDWEIGHTS stays hidden): V8[s,a] pairs = xT8 pairs.T @ wv8 pairs + bv;
    out[q,a] += P8_pair.T @ V8_pair. exp writes P directly as fp8 pairs
  - fp8 quantization noise on V/P is suppressed ~1/sqrt(n_keys) by the softmax
    average, so it is only visible for early query rows: rows < 256 (qb=0)
    instead use a small bf16 path (bf16 V chunks 0-1 + bf16 P + bf16 PV)
  - for q-half qs=0 the last key chunk of its q-block is fully masked; its
    exp()=0 fp8 plane rides the PV DoubleRow pair for free, and its score
    matmul only computes the live qs=1 half (N=128)
  - phase interleave: the kernel is DMA-bound for its first ~40us if all
    operands are loaded up front, so G s4-blocks, v-chains, early score
    blocks, and the V projections are emitted in the order their operands
    stream in from HBM (x/x8 column-block-major), keeping the PE busy from
    ~1us on. PV(qb) then trails ST(qb+k) with a deep pch ring
  - softmax denominator: VectorE accumulates P chunks into den_acc[128, QB]
    (fp32), one ones-rhs matmul pair per q-block reduces the partition dim;
    normalization is folded into the PSUM->SBUF output evacuation scale
  - every matmul accumulation chain owns a whole PSUM bank and runs
    uninterleaved (interleaved chains measurably stall the PE on this HW)
"""
import numpy as np

S = 2048
D = 1024
A = 1024
B = 8
QB = 256          # attention query-block width
NQB = S // QB     # 8
KC = 128          # attention key-chunk width
SCALE = 1.0 / 32.0  # 1/sqrt(A)
NEG = -1e30

_cache = {}


def _emit_body(nc, tc):
    import concourse.bass as bass
    import concourse.mybir as mybir

    f32 = mybir.dt.float32
    bf16 = mybir.dt.bfloat16
    fp8 = mybir.dt.float8e4
    DR = mybir.MatmulPerfMode.DoubleRow
    AF = mybir.ActivationFunctionType

    xTd, xT8d = nc.tensors["xT"], nc.tensors["xT8"]
    wmT, wvTd, wv8Td = nc.tensors["wmT"], nc.tensors["wvT"], nc.tensors["wv8T"]
    vcold, bvd = nc.tensors["vcol"], nc.tensors["bv"]
    mask, ones2, out = nc.tensors["mask"], nc.tensors["ones2"], nc.tensors["out"]

    def bcast_ap(handle, n_part, n_free):
        ap = handle[:]
        return bass.AP(tensor=ap.tensor, offset=ap.offset, ap=[[0, n_part], [1, n_free]])

    with (
        tc.tile_pool(name="const", bufs=1) as cp,
        tc.tile_pool(name="g", bufs=8) as gp,
        tc.tile_pool(name="v8", bufs=8) as v8p,
        tc.tile_pool(name="xt", bufs=1) as xtp,
        tc.tile_pool(name="pch", bufs=34) as pp,
        tc.tile_pool(name="pchb", bufs=2) as pbp,
        tc.tile_pool(name="stt", bufs=3) as sttp,
        tc.tile_pool(name="dac", bufs=8) as dap,
        tc.tile_pool(name="ob", bufs=4) as obp,
        tc.tile_pool(name="rin", bufs=4) as rp,
        tc.tile_pool(name="pst", bufs=3, space="PSUM") as pstp,
        tc.tile_pool(name="po", bufs=3, space="PSUM") as pop,
        tc.tile_pool(name="pd", bufs=1, space="PSUM") as pdp,
    ):
        g = [gp.tile([128, S], bf16, tag="g", name=f"g{i}") for i in range(8)]
        v8 = [v8p.tile([128, 2, A], fp8, tag="v8", name=f"v8_{i}") for i in range(8)]
        xt = xtp.tile([128, 8, S], bf16, tag="xt")
        xt8 = cp.tile([128, 8, S], fp8, tag="xt8")
        vb = [cp.tile([128, A], bf16, tag="vb0", name="vb0")]

        # DMA plan: the DMA engines are one shared ~360GB/s pipe and each
        # dma_start costs ~0.6us of HWDGE descriptor-gen regardless of size,
        # so every operand is ONE rearranged transfer, issued on one queue
        # (sync) in exact PE-consumption order. Narrow wm8 col-0 wave first
        # so the first G chain starts ~1us in.
        wm8 = cp.tile([128, 8, A], fp8, tag="wm8")
        wvb = cp.tile([128, 8, A], bf16, tag="wvb")
        wv8 = cp.tile([128, 8, A], fp8, tag="wv8")

        def r3(handle, lo, hi):
            return handle[:, lo:hi].rearrange("(d p) c -> p d c", p=128)

        nc.sync.dma_start(out=wm8[:, :, 0:128], in_=r3(wmT, 0, 128))
        nc.sync.dma_start(out=xt8[:, :, 0:512], in_=r3(xT8d, 0, 512))
        nc.sync.dma_start(out=wm8[:, :, 128:1024], in_=r3(wmT, 128, 1024))
        nc.sync.dma_start(out=xt[:, :, 0:512], in_=r3(xTd, 0, 512))
        nc.sync.dma_start(out=xt8[:, :, 512:1024], in_=r3(xT8d, 512, 1024))
        nc.sync.dma_start(out=xt[:, :, 512:1024], in_=r3(xTd, 512, 1024))
        nc.sync.dma_start(out=xt8[:, :, 1024:1536], in_=r3(xT8d, 1024, 1536))
        nc.sync.dma_start(out=xt[:, :, 1024:1536], in_=r3(xTd, 1024, 1536))
        nc.sync.dma_start(out=wvb, in_=r3(wvTd, 0, A))
        nc.sync.dma_start(out=xt8[:, :, 1536:2048], in_=r3(xT8d, 1536, 2048))
        nc.sync.dma_start(out=xt[:, :, 1536:2048], in_=r3(xTd, 1536, 2048))
        nc.sync.dma_start(out=wv8, in_=r3(wv8Td, 0, A))

        # consts on the gpsimd queue: vc first (v-chains run ~10us in),
        # bv_t last (first consumer is the V evacuation ~25us in)
        vc = cp.tile([128, 8], bf16, tag="vc")
        nc.gpsimd.dma_start(out=vc, in_=vcold.rearrange("(d p) -> p d", p=128))
        mk = [cp.tile([128, QB], f32, tag=f"mk{i}", name=f"mk{i}") for i in range(2)]
        for i in range(2):
            nc.gpsimd.dma_start(out=mk[i], in_=mask[i * 128 : (i + 1) * 128, :])
        ones_t = cp.tile([128, 2], f32, tag="ones")
        nc.gpsimd.dma_start(out=ones_t, in_=bcast_ap(ones2, 128, 2))
        bv_t = cp.tile([128, A], f32, tag="bv")
        nc.gpsimd.dma_start(out=bv_t, in_=bcast_ap(bvd, 128, A))
        # vsb[:, s_idx] = (1/32) * (x @ Wk^T @ bq) for key rows s_idx*128..+128
        vsb = cp.tile([128, 16], f32, tag="vsb")

        def g_block(s4):
            """G[d, s4*512:+512] = sum_d' MT[d',d] xT[d',s] (fp8 DoubleRow;
            MT is host-folded Wk^T Wq at x64 scale, undone on evacuation)."""
            for a in range(8):
                ps = pop.tile([128, 512], f32, tag="po")
                for d2 in range(4):
                    nc.tensor.matmul(
                        ps,
                        wm8[:, 2 * d2 : 2 * d2 + 2, a * 128 : (a + 1) * 128],
                        xt8[:, 2 * d2 : 2 * d2 + 2, s4 * 512 : (s4 + 1) * 512],
                        start=(d2 == 0),
                        stop=(d2 == 3),
                        perf_mode=DR,
                    )
                nc.scalar.activation(
                    g[a][:, s4 * 512 : (s4 + 1) * 512], ps, AF.Copy,
                    scale=1.0 / 64.0,
                )

        def v_block(blk):
            """vsb[:, s] = sum_d xT[d, s*128:+128]^T vcol[d] for the four
            key chunks of one 512-column block."""
            for s_idx in range(blk * 4, blk * 4 + 4):
                psv = pdp.tile([128, 2], f32, tag=f"pd{s_idx % 2}",
                               name=f"pd{s_idx % 2}")
                for d in range(8):
                    nc.tensor.matmul(
                        psv[:, 0:1],
                        xt[:, d, s_idx * 128 : (s_idx + 1) * 128],
                        vc[:, d : d + 1],
                        start=(d == 0),
                        stop=(d == 7),
                    )
                nc.vector.tensor_copy(vsb[:, s_idx : s_idx + 1], psv[:, 0:1])

        def vb_guard():
            """bf16 V for key chunk 0 (early-row guard) + fp8 copy."""
            for aq in range(4):
                ps = pstp.tile([128, QB], f32, tag="st")
                for d in range(8):
                    nc.tensor.matmul(
                        ps,
                        xt[:, d, 0:128],
                        wvb[:, d, aq * 256 : (aq + 1) * 256],
                        start=(d == 0),
                        stop=(d == 7),
                    )
                nc.vector.tensor_add(
                    vb[0][:, aq * 256 : (aq + 1) * 256],
                    ps,
                    bv_t[:, aq * 256 : (aq + 1) * 256],
                )
            nc.vector.tensor_copy(v8[0][:, 0, :], vb[0])

        def v8_block():
            """fp8 DoubleRow V for key chunks 1-15."""
            for s_idx in range(1, 16):
                for aq2 in range(2):
                    ps = pop.tile([128, 512], f32, tag="po")
                    for d2 in range(4):
                        nc.tensor.matmul(
                            ps,
                            xt8[:, 2 * d2 : 2 * d2 + 2,
                                s_idx * 128 : (s_idx + 1) * 128],
                            wv8[:, 2 * d2 : 2 * d2 + 2,
                                aq2 * 512 : (aq2 + 1) * 512],
                            start=(d2 == 0),
                            stop=(d2 == 3),
                            perf_mode=DR,
                        )
                    nc.vector.tensor_add(
                        v8[s_idx // 2][:, s_idx % 2, aq2 * 512 : (aq2 + 1) * 512],
                        ps,
                        bv_t[:, aq2 * 512 : (aq2 + 1) * 512],
                    )

        def emit_st(qb):
            """Score chunks + exp + denominator partials for one q-block.

            Returns (pch_handles, den_acc): for qb==0 pch_handles are two
            bf16 [128,QB] tiles; for qb>=1 they are fp8 pair tiles
            [128,2,QB], one per key-chunk pair.
            """
            nkc = (qb + 1) * QB // KC
            den_acc = dap.tile([128, QB], f32, tag="dac", name=f"dac{qb%8}")
            pchs = []
            if qb == 0:
                # guard block: chunk 0 exp'd twice (bf16 for the qs=0 rows
                # <128 guard path, fp8 plane for the qs=1 fp8 path); chunk 1
                # narrow (qs=0 half fully masked)
                pch0 = pbp.tile([128, QB], bf16, tag="pb", name="pb0")
                pair0 = pp.tile([128, 2, QB], fp8, tag="p", name="p0")
                ps = pstp.tile([128, QB], f32, tag="st")
                for a in range(8):
                    nc.tensor.matmul(
                        ps,
                        g[a][:, 0:KC],
                        xt[:, a, 0:QB],
                        start=(a == 0),
                        stop=(a == 7),
                    )
                stt = sttp.tile([128, QB], f32, tag="stt")
                nc.vector.tensor_add(stt, ps, mk[0])
                vbias = vsb[:, 0:1]
                nc.scalar.activation(pch0[:, 0:128], stt[:, 0:128], AF.Exp,
                                     scale=SCALE, bias=vbias)
                nc.scalar.activation(pair0[:, 0, :], stt, AF.Exp,
                                     scale=SCALE, bias=vbias)
                nc.vector.tensor_copy(den_acc[:, 0:128], pch0[:, 0:128])
                nc.vector.tensor_copy(den_acc[:, 128:QB], pair0[:, 0, 128:QB])
                ps = pstp.tile([128, QB], f32, tag="st")
                for a in range(8):
                    nc.tensor.matmul(
                        ps[:, 128:QB],
                        g[a][:, KC : 2 * KC],
                        xt[:, a, 128:QB],
                        start=(a == 0),
                        stop=(a == 7),
                    )
                nc.vector.memset(pair0[:, 1, 0:128], 0.0)
                stt = sttp.tile([128, QB], f32, tag="stt")
                nc.vector.tensor_add(stt[:, 0:128], ps[:, 128:QB], mk[1][:, 128:QB])
                nc.scalar.activation(pair0[:, 1, 128:QB], stt[:, 0:128], AF.Exp,
                                     scale=SCALE, bias=vsb[:, 1:2])
                nc.vector.tensor_add(
                    den_acc[:, 128:QB], den_acc[:, 128:QB], pair0[:, 1, 128:QB]
                )
                return [pch0, pair0], den_acc
            for kc in range(nkc):
                vbias = vsb[:, kc : kc + 1]
                if False:
                    pass
                else:
                    if kc % 2 == 0:
                        pair = pp.tile([128, 2, QB], fp8, tag="p",
                                       name=f"p{(kc // 2) % 34}")
                        pchs.append(pair)
                    dst = pchs[-1][:, kc % 2, :]
                if qb > 0 and kc == 2 * qb + 1:
                    # top diagonal chunk: the qs=0 query half is fully masked -
                    # compute scores only for the qs=1 half and zero the dead
                    # half of the fp8 plane (it still rides the PV pair)
                    ps = pstp.tile([128, QB], f32, tag="st")
                    for a in range(8):
                        nc.tensor.matmul(
                            ps[:, 128:QB],
                            g[a][:, kc * KC : (kc + 1) * KC],
                            xt[:, a, qb * QB + 128 : (qb + 1) * QB],
                            start=(a == 0),
                            stop=(a == 7),
                        )
                    nc.vector.memset(dst[:, 0:128], 0.0)
                    stt = sttp.tile([128, QB], f32, tag="stt")
                    nc.vector.tensor_add(
                        stt[:, 0:128], ps[:, 128:QB], mk[1][:, 128:QB]
                    )
                    nc.scalar.activation(
                        dst[:, 128:QB], stt[:, 0:128], AF.Exp,
                        scale=SCALE, bias=vbias,
                    )
                    nc.vector.tensor_add(
                        den_acc[:, 128:QB], den_acc[:, 128:QB], dst[:, 128:QB]
                    )
                    continue
                ps = pstp.tile([128, QB], f32, tag="st")
                for a in range(8):
                    nc.tensor.matmul(
                        ps,
                        g[a][:, kc * KC : (kc + 1) * KC],
                        xt[:, a, qb * QB : (qb + 1) * QB],
                        start=(a == 0),
                        stop=(a == 7),
                    )
                if kc >= 2 * qb:  # diagonal 256x256 block: apply causal mask
                    mrow = kc - 2 * qb
                    stt = sttp.tile([128, QB], f32, tag="stt")
                    nc.vector.tensor_add(stt, ps, mk[mrow])
                    nc.scalar.activation(dst, stt, AF.Exp, scale=SCALE, bias=vbias)
                else:
                    nc.scalar.activation(dst, ps, AF.Exp, scale=SCALE, bias=vbias)
                if kc == 0:
                    nc.vector.tensor_copy(den_acc, dst)
                else:
                    nc.vector.tensor_add(den_acc, den_acc, dst)
            return pchs, den_acc

        def emit_pv(qb, pchs, den_acc):
            """Denominator reduction + PV chains + evacuation for one q-block."""
            pd = [
                pdp.tile([128, 2], f32, tag=f"pd{i}", name=f"pd{i}")
                for i in range(2)
            ]
            rinvs = []
            for qs in range(2):
                nc.tensor.matmul(
                    pd[qs][:, 0:2],
                    den_acc[:, qs * 128 : (qs + 1) * 128],
                    ones_t,
                    start=True,
                    stop=True,
                )
                rinv = rp.tile([128, 1], f32, tag="rinv")
                nc.vector.reciprocal(rinv, pd[qs][:, 0:1])
                rinvs.append(rinv)
            osbs = [obp.tile([128, A], f32, tag="ob", name=f"ob{i}") for i in range(2)]
            if qb == 0:
                # guard path: rows <128 via bf16 chunk 0, rows 128-255 via the
                # fp8 pair (chunk 1's dead half is the zeroed plane region)
                for aq in range(4):
                    dst = pop.tile([128, 512], f32, tag="po")
                    nc.tensor.matmul(
                        dst[:, 0:256],
                        pchs[0][:, 0:128],
                        vb[0][:, aq * 256 : (aq + 1) * 256],
                        start=True,
                        stop=True,
                    )
                    nc.scalar.activation(
                        osbs[0][:, aq * 256 : (aq + 1) * 256],
                        dst[:, 0:256],
                        AF.Copy,
                        scale=rinvs[0],
                    )
                    nc.sync.dma_start(
                        out=out[0:128, aq * 256 : (aq + 1) * 256],
                        in_=osbs[0][:, aq * 256 : (aq + 1) * 256],
                    )
                for aq2 in range(2):
                    dst = pop.tile([128, 512], f32, tag="po")
                    nc.tensor.matmul(
                        dst,
                        pchs[1][:, 0:2, 128:QB],
                        v8[0][:, 0:2, aq2 * 512 : (aq2 + 1) * 512],
                        start=True,
                        stop=True,
                        perf_mode=DR,
                    )
                    nc.scalar.activation(
                        osbs[1][:, aq2 * 512 : (aq2 + 1) * 512],
                        dst,
                        AF.Copy,
                        scale=rinvs[1],
                    )
                    nc.sync.dma_start(
                        out=out[128:256, aq2 * 512 : (aq2 + 1) * 512],
                        in_=osbs[1][:, aq2 * 512 : (aq2 + 1) * 512],
                    )
                return
            npair = qb + 1
            for qs in range(2):
                for aq2 in range(2):
                    dst = pop.tile([128, 512], f32, tag="po")
                    for kp in range(npair):
                        nc.tensor.matmul(
                            dst,
                            pchs[kp][:, 0:2, qs * 128 : qs * 128 + 128],
                            v8[kp][:, 0:2, aq2 * 512 : (aq2 + 1) * 512],
                            start=(kp == 0),
                            stop=(kp == npair - 1),
                            perf_mode=DR,
                        )
                    nc.scalar.activation(
                        osbs[qs][:, aq2 * 512 : (aq2 + 1) * 512],
                        dst,
                        AF.Copy,
                        scale=rinvs[qs],
                    )
                    row = qb * QB + qs * 128
                    nc.sync.dma_start(
                        out=out[row : row + 128, aq2 * 512 : (aq2 + 1) * 512],
                        in_=osbs[qs][:, aq2 * 512 : (aq2 + 1) * 512],
                    )

        # phase interleave, paced to the x/x8 column-block DMA stream;
        # st(qb) needs G columns <= (2qb+2)*128 and x columns <= (qb+1)*256
        sts = []
        g_block(0)
        v_block(0)
        sts.append(emit_st(0))
        sts.append(emit_st(1))
        g_block(1)
        v_block(1)
        sts.append(emit_st(2))
        sts.append(emit_st(3))
        g_block(2)
        v_block(2)
        sts.append(emit_st(4))
        vb_guard()
        g_block(3)
        v_block(3)
        v8_block()
        for qb in range(5, NQB):
            emit_pv(qb - 5, *sts[qb - 5])
            sts.append(emit_st(qb))
        for qb in range(NQB - 5, NQB):
            emit_pv(qb, *sts[qb])


def _build(repeat=1):
    from concourse import bacc
    import concourse.mybir as mybir
    import concourse.tile as tile

    f32 = mybir.dt.float32
    bf16 = mybir.dt.bfloat16
    fp8 = mybir.dt.float8e4

    nc = bacc.Bacc("TRN2", target_bir_lowering=False)
    nc.tensors = {}
    nc.tensors["xT"] = nc.dram_tensor("xT", [D, S], bf16, kind="ExternalInput")
    nc.tensors["xT8"] = nc.dram_tensor("xT8", [D, S], fp8, kind="ExternalInput")
    nc.tensors["wmT"] = nc.dram_tensor("wmT", [D, D], fp8, kind="ExternalInput")
    nc.tensors["wvT"] = nc.dram_tensor("wvT", [D, A], bf16, kind="ExternalInput")
    nc.tensors["wv8T"] = nc.dram_tensor("wv8T", [D, A], fp8, kind="ExternalInput")
    nc.tensors["vcol"] = nc.dram_tensor("vcol", [D], bf16, kind="ExternalInput")
    nc.tensors["bv"] = nc.dram_tensor("bv", [A], f32, kind="ExternalInput")
    nc.tensors["mask"] = nc.dram_tensor("mask", [QB, QB], f32, kind="ExternalInput")
    nc.tensors["ones2"] = nc.dram_tensor("ones2", [2], f32, kind="ExternalInput")
    nc.tensors["out"] = nc.dram_tensor("out", [S, A], f32, kind="ExternalOutput")

    with tile.TileContext(nc) as tc:
        if repeat > 1:
            with tc.For_i(0, repeat, 1):
                _emit_body(nc, tc)
        else:
            _emit_body(nc, tc)

    nc.finalize()
    return nc


def _prep_in_maps(x, Wq, bq, Wk, bk, Wv, bv):
    """Build per-core input maps (host-side shard + weight-folding/layout)."""
    import ml_dtypes

    bf = ml_dtypes.bfloat16
    f8 = ml_dtypes.float8_e4m3
    x = np.asarray(x, dtype=np.float32)
    Wq = np.asarray(Wq, dtype=np.float64)
    Wk = np.asarray(Wk, dtype=np.float64)
    bq = np.asarray(bq, dtype=np.float64)
    # MT[d',d] = (Wk^T Wq)[d',d] = M[d,d'] with M = Wq^T Wk (weight folding);
    # stored fp8 at x64 scale to clear the e4m3 subnormal threshold
    wmT = np.ascontiguousarray((64.0 * (Wk.T @ Wq)).astype(np.float32).astype(f8))
    wvT32 = np.asarray(Wv, dtype=np.float32).T
    wvT = np.ascontiguousarray(wvT32.astype(bf))
    wv8T = np.ascontiguousarray(wvT32.astype(f8))
    vcol = (SCALE * (Wk.T @ bq)).astype(np.float32).astype(bf)
    bv = np.asarray(bv, dtype=np.float32)
    kq = np.arange(QB)
    mask = np.where(kq[:, None] <= kq[None, :], 0.0, NEG).astype(np.float32)
    ones2 = np.ones(2, dtype=np.float32)
    in_maps = []
    for c in range(B):
        xT32 = np.ascontiguousarray(x[:, c, :].T)
        in_maps.append(
            {
                "xT": xT32.astype(bf), "xT8": xT32.astype(f8),
                "wmT": wmT, "wvT": wvT, "wv8T": wv8T, "vcol": vcol,
                "bv": bv, "mask": mask, "ones2": ones2,
            }
        )
    return in_maps


def get_nc(repeat=1):
    key = ("nc", repeat)
    if key not in _cache:
        _cache[key] = _build(repeat)
    return _cache[key]


def kernel(x, Wq, bq, Wk, bk, Wv, bv):
    from concourse.bass_utils import run_bass_kernel_spmd

    nc = get_nc()
    in_maps = _prep_in_maps(x, Wq, bq, Wk, bk, Wv, bv)
    res = run_bass_kernel_spmd(nc, in_maps, core_ids=list(range(B)))
    outs = np.stack([res.results[c]["out"] for c in range(B)], axis=0)  # (B, S, A)
    return np.ascontiguousarray(outs.transpose(1, 0, 2))  # (S, B, A)


# revision 43
# speedup vs baseline: 1.0280x; 1.0280x over previous
"""Causal single-head attention on 8 trn2 NeuronCores, data-parallel over batch.

Reference computation (per batch element b):
  Q = x_b @ Wq.T + bq ; K = x_b @ Wk.T + bk ; V = x_b @ Wv.T + bv    (S=2048, D=A=1024)
  out_b = softmax(causal(Q K^T / 32)) V

Sharding: x is (S, B=8, D); core c handles batch element c. No collectives.

Per-core kernel design (v6: folded scores, fp8 DoubleRow G/V/PV, DMA-paced
phase interleave):
  - softmax is shift-invariant per row, so the q-side bias terms of
    (Q+bq)(K+bk)^T cancel; scores reduce to x M x^T + 1 v^T with M = Wq^T Wk
    folded on the HOST (weights-only folding) and v = x (Wk^T bq) computed on
    device (16 small PE matvec chains -> vsb[128,16], one column per key
    chunk). Removes both Q and K projections (-19% MACs vs the v2 baseline)
  - G pass in fp8 DoubleRow (M scaled x64 to clear the e4m3 subnormal floor,
    undone in the PSUM evacuation scale); the score matmul itself stays bf16:
    quantizing its operands too was measured at rel_err ~2e-2 = the gate
  - ST[k,q] = G_chunk.T @ xT_block; exp(SCALE*ps + vsb[kc]) on ScalarE (the
    k-side bias rides the per-partition activation bias slot; no row-max
    subtraction, |args| < ~2.5)
  - V projection and P@V run in fp8e4m3 with perf_mode=DoubleRow (2 fp8
    contraction planes per PE cell, moving free dim 512 so the doubled
    LDWEIGHTS stays hidden): V8[s,a] pairs = xT8 pairs.T @ wv8 pairs + bv;
    out[q,a] += P8_pair.T @ V8_pair. exp writes P directly as fp8 pairs
  - fp8 quantization noise on V/P is suppressed ~1/sqrt(n_keys) by the softmax
    average, so it is only visible for early query rows: rows < 128 instead
    use a small bf16 path (bf16 V chunk 0 + bf16 P + bf16 PV)
  - for q-half qs=0 the last key chunk of its q-block is fully masked; its
    exp()=0 fp8 plane rides the PV DoubleRow pair for free, and its score
    matmul only computes the live qs=1 half (N=128)
  - phase interleave: the kernel is DMA-bound for its first ~40us if all
    operands are loaded up front, so G s4-blocks, v-chains, early score
    blocks, and the V projections are emitted in the order their operands
    stream in from HBM (x/x8 column-block-major), keeping the PE busy from
    ~1us on. PV(qb) then trails ST(qb+k) with a deep pch ring
  - softmax denominator: VectorE accumulates P chunks into den_acc[128, QB]
    (fp32), one ones-rhs matmul pair per q-block reduces the partition dim;
    normalization is folded into the PSUM->SBUF output evacuation scale
  - every matmul accumulation chain owns a whole PSUM bank and runs
    uninterleaved (interleaved chains measurably stall the PE on this HW)
"""
import numpy as np

S = 2048
D = 1024
A = 1024
B = 8
QB = 256          # attention query-block width
NQB = S // QB     # 8
KC = 128          # attention key-chunk width
SCALE = 1.0 / 32.0  # 1/sqrt(A)
NEG = -1e30

_cache = {}


def _emit_body(nc, tc):
    import concourse.bass as bass
    import concourse.mybir as mybir

    f32 = mybir.dt.float32
    bf16 = mybir.dt.bfloat16
    fp8 = mybir.dt.float8e4
    DR = mybir.MatmulPerfMode.DoubleRow
    AF = mybir.ActivationFunctionType

    xTd, xT8d = nc.tensors["xT"], nc.tensors["xT8"]
    wmT, wvTd, wv8Td = nc.tensors["wmT"], nc.tensors["wvT"], nc.tensors["wv8T"]
    vcold, bvd = nc.tensors["vcol"], nc.tensors["bv"]
    mask, ones2, out = nc.tensors["mask"], nc.tensors["ones2"], nc.tensors["out"]

    def bcast_ap(handle, n_part, n_free):
        ap = handle[:]
        return bass.AP(tensor=ap.tensor, offset=ap.offset, ap=[[0, n_part], [1, n_free]])

    with (
        tc.tile_pool(name="const", bufs=1) as cp,
        tc.tile_pool(name="g", bufs=8) as gp,
        tc.tile_pool(name="v8", bufs=8) as v8p,
        tc.tile_pool(name="xt", bufs=1) as xtp,
        tc.tile_pool(name="pch", bufs=34) as pp,
        tc.tile_pool(name="pchb", bufs=2) as pbp,
        tc.tile_pool(name="stt", bufs=3) as sttp,
        tc.tile_pool(name="dac", bufs=8) as dap,
        tc.tile_pool(name="ob", bufs=4) as obp,
        tc.tile_pool(name="rin", bufs=4) as rp,
        tc.tile_pool(name="pst", bufs=3, space="PSUM") as pstp,
        tc.tile_pool(name="po", bufs=3, space="PSUM") as pop,
        tc.tile_pool(name="pd", bufs=1, space="PSUM") as pdp,
    ):
        g = [gp.tile([128, S], bf16, tag="g", name=f"g{i}") for i in range(8)]
        v8 = [v8p.tile([128, 2, A], fp8, tag="v8", name=f"v8_{i}") for i in range(8)]
        xt = xtp.tile([128, 8, S], bf16, tag="xt")
        xt8 = cp.tile([128, 8, S], fp8, tag="xt8")
        vb = [cp.tile([128, A], bf16, tag="vb0", name="vb0")]

        # DMA plan: the DMA engines are one shared ~360GB/s pipe and each
        # dma_start costs ~0.6us of HWDGE descriptor-gen regardless of size,
        # so every operand is ONE rearranged transfer, issued on one queue
        # (sync) in exact PE-consumption order. Narrow wm8 col-0 wave first
        # so the first G chain starts ~1us in.
        wm8 = cp.tile([128, 8, A], fp8, tag="wm8")
        wvb = cp.tile([128, 8, A], bf16, tag="wvb")
        wv8 = cp.tile([128, 8, A], fp8, tag="wv8")

        def r3(handle, lo, hi):
            return handle[:, lo:hi].rearrange("(d p) c -> p d c", p=128)

        nc.sync.dma_start(out=wm8[:, :, 0:128], in_=r3(wmT, 0, 128))
        nc.sync.dma_start(out=xt8[:, :, 0:512], in_=r3(xT8d, 0, 512))
        nc.sync.dma_start(out=wm8[:, :, 128:1024], in_=r3(wmT, 128, 1024))
        nc.sync.dma_start(out=xt[:, :, 0:512], in_=r3(xTd, 0, 512))
        nc.sync.dma_start(out=xt8[:, :, 512:1024], in_=r3(xT8d, 512, 1024))
        nc.sync.dma_start(out=xt[:, :, 512:1024], in_=r3(xTd, 512, 1024))
        nc.sync.dma_start(out=xt8[:, :, 1024:1536], in_=r3(xT8d, 1024, 1536))
        nc.sync.dma_start(out=xt[:, :, 1024:1536], in_=r3(xTd, 1024, 1536))
        nc.sync.dma_start(out=wvb, in_=r3(wvTd, 0, A))
        nc.sync.dma_start(out=xt8[:, :, 1536:2048], in_=r3(xT8d, 1536, 2048))
        nc.sync.dma_start(out=xt[:, :, 1536:2048], in_=r3(xTd, 1536, 2048))
        nc.sync.dma_start(out=wv8, in_=r3(wv8Td, 0, A))

        # consts on the gpsimd queue: vc first (v-chains run ~10us in),
        # bv_t last (first consumer is the V evacuation ~25us in)
        vc = cp.tile([128, 8], bf16, tag="vc")
        nc.gpsimd.dma_start(out=vc, in_=vcold.rearrange("(d p) -> p d", p=128))
        mk = [cp.tile([128, QB], f32, tag=f"mk{i}", name=f"mk{i}") for i in range(2)]
        for i in range(2):
            nc.gpsimd.dma_start(out=mk[i], in_=mask[i * 128 : (i + 1) * 128, :])
        ones_t = cp.tile([128, 2], f32, tag="ones")
        nc.gpsimd.dma_start(out=ones_t, in_=bcast_ap(ones2, 128, 2))
        bv_t = cp.tile([128, A], f32, tag="bv")
        nc.gpsimd.dma_start(out=bv_t, in_=bcast_ap(bvd, 128, A))
        # vsb[:, s_idx] = (1/32) * (x @ Wk^T @ bq) for key rows s_idx*128..+128
        vsb = cp.tile([128, 16], f32, tag="vsb")

        def g_block(s4):
            """G[d, s4*512:+512] = sum_d' MT[d',d] xT[d',s] (fp8 DoubleRow;
            MT is host-folded Wk^T Wq at x64 scale, undone on evacuation)."""
            for a in range(8):
                ps = pop.tile([128, 512], f32, tag="po")
                for d2 in range(4):
                    nc.tensor.matmul(
                        ps,
                        wm8[:, 2 * d2 : 2 * d2 + 2, a * 128 : (a + 1) * 128],
                        xt8[:, 2 * d2 : 2 * d2 + 2, s4 * 512 : (s4 + 1) * 512],
                        start=(d2 == 0),
                        stop=(d2 == 3),
                        perf_mode=DR,
                    )
                nc.scalar.activation(
                    g[a][:, s4 * 512 : (s4 + 1) * 512], ps, AF.Copy,
                    scale=1.0 / 64.0,
                )

        def v_block(blk):
            """vsb[:, s] = sum_d xT[d, s*128:+128]^T vcol[d] for the four
            key chunks of one 512-column block."""
            for s_idx in range(blk * 4, blk * 4 + 4):
                psv = pdp.tile([128, 2], f32, tag=f"pd{s_idx % 2}",
                               name=f"pd{s_idx % 2}")
                for d in range(8):
                    nc.tensor.matmul(
                        psv[:, 0:1],
                        xt[:, d, s_idx * 128 : (s_idx + 1) * 128],
                        vc[:, d : d + 1],
                        start=(d == 0),
                        stop=(d == 7),
                    )
                nc.vector.tensor_copy(vsb[:, s_idx : s_idx + 1], psv[:, 0:1])

        def vb_guard():
            """bf16 V for key chunk 0 (early-row guard) + fp8 copy."""
            for aq in range(4):
                ps = pstp.tile([128, QB], f32, tag="st")
                for d in range(8):
                    nc.tensor.matmul(
                        ps,
                        xt[:, d, 0:128],
                        wvb[:, d, aq * 256 : (aq + 1) * 256],
                        start=(d == 0),
                        stop=(d == 7),
                    )
                nc.vector.tensor_add(
                    vb[0][:, aq * 256 : (aq + 1) * 256],
                    ps,
                    bv_t[:, aq * 256 : (aq + 1) * 256],
                )
            nc.vector.tensor_copy(v8[0][:, 0, :], vb[0])

        def v8_block():
            """fp8 DoubleRow V for key chunks 1-15."""
            for s_idx in range(1, 16):
                for aq2 in range(2):
                    ps = pop.tile([128, 512], f32, tag="po")
                    for d2 in range(4):
                        nc.tensor.matmul(
                            ps,
                            xt8[:, 2 * d2 : 2 * d2 + 2,
                                s_idx * 128 : (s_idx + 1) * 128],
                            wv8[:, 2 * d2 : 2 * d2 + 2,
                                aq2 * 512 : (aq2 + 1) * 512],
                            start=(d2 == 0),
                            stop=(d2 == 3),
                            perf_mode=DR,
                        )
                    nc.vector.tensor_add(
                        v8[s_idx // 2][:, s_idx % 2, aq2 * 512 : (aq2 + 1) * 512],
                        ps,
                        bv_t[:, aq2 * 512 : (aq2 + 1) * 512],
                    )

        def emit_st(qb):
            """Score chunks + exp + denominator partials for one q-block.

            Returns (pch_handles, den_acc): for qb==0 pch_handles are two
            bf16 [128,QB] tiles; for qb>=1 they are fp8 pair tiles
            [128,2,QB], one per key-chunk pair.
            """
            nkc = (qb + 1) * QB // KC
            den_acc = dap.tile([128, QB], f32, tag="dac", name=f"dac{qb%8}")
            pchs = []
            if qb == 0:
                # guard block: chunk 0 exp'd twice (bf16 for the qs=0 rows
                # <128 guard path, fp8 plane for the qs=1 fp8 path); chunk 1
                # narrow (qs=0 half fully masked)
                pch0 = pbp.tile([128, QB], bf16, tag="pb", name="pb0")
                pair0 = pp.tile([128, 2, QB], fp8, tag="p", name="p0")
                ps = pstp.tile([128, QB], f32, tag="st")
                for a in range(8):
                    nc.tensor.matmul(
                        ps,
                        g[a][:, 0:KC],
                        xt[:, a, 0:QB],
                        start=(a == 0),
                        stop=(a == 7),
                    )
                stt = sttp.tile([128, QB], f32, tag="stt")
                nc.vector.tensor_add(stt, ps, mk[0])
                vbias = vsb[:, 0:1]
                nc.scalar.activation(pch0[:, 0:128], stt[:, 0:128], AF.Exp,
                                     scale=SCALE, bias=vbias)
                nc.scalar.activation(pair0[:, 0, :], stt, AF.Exp,
                                     scale=SCALE, bias=vbias)
                nc.vector.tensor_copy(den_acc[:, 0:128], pch0[:, 0:128])
                nc.vector.tensor_copy(den_acc[:, 128:QB], pair0[:, 0, 128:QB])
                ps = pstp.tile([128, QB], f32, tag="st")
                for a in range(8):
                    nc.tensor.matmul(
                        ps[:, 128:QB],
                        g[a][:, KC : 2 * KC],
                        xt[:, a, 128:QB],
                        start=(a == 0),
                        stop=(a == 7),
                    )
                nc.vector.memset(pair0[:, 1, 0:128], 0.0)
                stt = sttp.tile([128, QB], f32, tag="stt")
                nc.vector.tensor_add(stt[:, 0:128], ps[:, 128:QB], mk[1][:, 128:QB])
                nc.scalar.activation(pair0[:, 1, 128:QB], stt[:, 0:128], AF.Exp,
                                     scale=SCALE, bias=vsb[:, 1:2])
                nc.vector.tensor_add(
                    den_acc[:, 128:QB], den_acc[:, 128:QB], pair0[:, 1, 128:QB]
                )
                return [pch0, pair0], den_acc
            for kc in range(nkc):
                vbias = vsb[:, kc : kc + 1]
                if kc % 2 == 0:
                    pair = pp.tile([128, 2, QB], fp8, tag="p",
                                   name=f"p{(kc // 2) % 34}")
                    pchs.append(pair)
                dst = pchs[-1][:, kc % 2, :]
                if qb > 0 and kc == 2 * qb + 1:
                    # top diagonal chunk: the qs=0 query half is fully masked -
                    # compute scores only for the qs=1 half and zero the dead
                    # half of the fp8 plane (it still rides the PV pair)
                    ps = pstp.tile([128, QB], f32, tag="st")
                    for a in range(8):
                        nc.tensor.matmul(
                            ps[:, 128:QB],
                            g[a][:, kc * KC : (kc + 1) * KC],
                            xt[:, a, qb * QB + 128 : (qb + 1) * QB],
                            start=(a == 0),
                            stop=(a == 7),
                        )
                    nc.vector.memset(dst[:, 0:128], 0.0)
                    stt = sttp.tile([128, QB], f32, tag="stt")
                    nc.vector.tensor_add(
                        stt[:, 0:128], ps[:, 128:QB], mk[1][:, 128:QB]
                    )
                    nc.scalar.activation(
                        dst[:, 128:QB], stt[:, 0:128], AF.Exp,
                        scale=SCALE, bias=vbias,
                    )
                    nc.vector.tensor_add(
                        den_acc[:, 128:QB], den_acc[:, 128:QB], dst[:, 128:QB]
                    )
                    continue
                ps = pstp.tile([128, QB], f32, tag="st")
                for a in range(8):
                    nc.tensor.matmul(
                        ps,
                        g[a][:, kc * KC : (kc + 1) * KC],
                        xt[:, a, qb * QB : (qb + 1) * QB],
                        start=(a == 0),
                        stop=(a == 7),
                    )
                if kc >= 2 * qb:  # diagonal 256x256 block: apply causal mask
                    mrow = kc - 2 * qb
                    stt = sttp.tile([128, QB], f32, tag="stt")
                    nc.vector.tensor_add(stt, ps, mk[mrow])
                    nc.scalar.activation(dst, stt, AF.Exp, scale=SCALE, bias=vbias)
                else:
                    nc.scalar.activation(dst, ps, AF.Exp, scale=SCALE, bias=vbias)
                if kc == 0:
                    nc.vector.tensor_copy(den_acc, dst)
                else:
                    nc.vector.tensor_add(den_acc, den_acc, dst)
            return pchs, den_acc

        def emit_pv(qb, pchs, den_acc):
            """Denominator reduction + PV chains + evacuation for one q-block."""
            pd = [
                pdp.tile([128, 2], f32, tag=f"pd{i}", name=f"pd{i}")
                for i in range(2)
            ]
            rinvs = []
            for qs in range(2):
                nc.tensor.matmul(
                    pd[qs][:, 0:2],
                    den_acc[:, qs * 128 : (qs + 1) * 128],
                    ones_t,
                    start=True,
                    stop=True,
                )
                rinv = rp.tile([128, 1], f32, tag="rinv")
                nc.vector.reciprocal(rinv, pd[qs][:, 0:1])
                rinvs.append(rinv)
            osbs = [obp.tile([128, A], f32, tag="ob", name=f"ob{i}") for i in range(2)]
            if qb == 0:
                # guard path: rows <128 via bf16 chunk 0, rows 128-255 via the
                # fp8 pair (chunk 1's dead half is the zeroed plane region)
                for aq in range(4):
                    dst = pop.tile([128, 512], f32, tag="po")
                    nc.tensor.matmul(
                        dst[:, 0:256],
                        pchs[0][:, 0:128],
                        vb[0][:, aq * 256 : (aq + 1) * 256],
                        start=True,
                        stop=True,
                    )
                    nc.scalar.activation(
                        osbs[0][:, aq * 256 : (aq + 1) * 256],
                        dst[:, 0:256],
                        AF.Copy,
                        scale=rinvs[0],
                    )
                    nc.sync.dma_start(
                        out=out[0:128, aq * 256 : (aq + 1) * 256],
                        in_=osbs[0][:, aq * 256 : (aq + 1) * 256],
                    )
                for aq2 in range(2):
                    dst = pop.tile([128, 512], f32, tag="po")
                    nc.tensor.matmul(
                        dst,
                        pchs[1][:, 0:2, 128:QB],
                        v8[0][:, 0:2, aq2 * 512 : (aq2 + 1) * 512],
                        start=True,
                        stop=True,
                        perf_mode=DR,
                    )
                    nc.scalar.activation(
                        osbs[1][:, aq2 * 512 : (aq2 + 1) * 512],
                        dst,
                        AF.Copy,
                        scale=rinvs[1],
                    )
                    nc.sync.dma_start(
                        out=out[128:256, aq2 * 512 : (aq2 + 1) * 512],
                        in_=osbs[1][:, aq2 * 512 : (aq2 + 1) * 512],
                    )
                return
            npair = qb + 1
            for qs in range(2):
                for aq2 in range(2):
                    dst = pop.tile([128, 512], f32, tag="po")
                    for kp in range(npair):
                        nc.tensor.matmul(
                            dst,
                            pchs[kp][:, 0:2, qs * 128 : qs * 128 + 128],
                            v8[kp][:, 0:2, aq2 * 512 : (aq2 + 1) * 512],
                            start=(kp == 0),
                            stop=(kp == npair - 1),
                            perf_mode=DR,
                        )
                    nc.scalar.activation(
                        osbs[qs][:, aq2 * 512 : (aq2 + 1) * 512],
                        dst,
                        AF.Copy,
                        scale=rinvs[qs],
                    )
                    row = qb * QB + qs * 128
                    nc.sync.dma_start(
                        out=out[row : row + 128, aq2 * 512 : (aq2 + 1) * 512],
                        in_=osbs[qs][:, aq2 * 512 : (aq2 + 1) * 512],
                    )

        # phase interleave, paced to the x/x8 column-block DMA stream;
        # st(qb) needs G columns <= (2qb+2)*128 and x columns <= (qb+1)*256
        sts = []
        g_block(0)
        v_block(0)
        sts.append(emit_st(0))
        sts.append(emit_st(1))
        g_block(1)
        v_block(1)
        sts.append(emit_st(2))
        sts.append(emit_st(3))
        g_block(2)
        v_block(2)
        sts.append(emit_st(4))
        vb_guard()
        g_block(3)
        v_block(3)
        v8_block()
        for qb in range(5, NQB):
            emit_pv(qb - 5, *sts[qb - 5])
            sts.append(emit_st(qb))
        for qb in range(NQB - 5, NQB):
            emit_pv(qb, *sts[qb])


def _build(repeat=1):
    from concourse import bacc
    import concourse.mybir as mybir
    import concourse.tile as tile

    f32 = mybir.dt.float32
    bf16 = mybir.dt.bfloat16
    fp8 = mybir.dt.float8e4

    nc = bacc.Bacc("TRN2", target_bir_lowering=False)
    nc.tensors = {}
    nc.tensors["xT"] = nc.dram_tensor("xT", [D, S], bf16, kind="ExternalInput")
    nc.tensors["xT8"] = nc.dram_tensor("xT8", [D, S], fp8, kind="ExternalInput")
    nc.tensors["wmT"] = nc.dram_tensor("wmT", [D, D], fp8, kind="ExternalInput")
    nc.tensors["wvT"] = nc.dram_tensor("wvT", [D, A], bf16, kind="ExternalInput")
    nc.tensors["wv8T"] = nc.dram_tensor("wv8T", [D, A], fp8, kind="ExternalInput")
    nc.tensors["vcol"] = nc.dram_tensor("vcol", [D], bf16, kind="ExternalInput")
    nc.tensors["bv"] = nc.dram_tensor("bv", [A], f32, kind="ExternalInput")
    nc.tensors["mask"] = nc.dram_tensor("mask", [QB, QB], f32, kind="ExternalInput")
    nc.tensors["ones2"] = nc.dram_tensor("ones2", [2], f32, kind="ExternalInput")
    nc.tensors["out"] = nc.dram_tensor("out", [S, A], f32, kind="ExternalOutput")

    with tile.TileContext(nc) as tc:
        if repeat > 1:
            with tc.For_i(0, repeat, 1):
                _emit_body(nc, tc)
        else:
            _emit_body(nc, tc)

    nc.finalize()
    return nc


def _prep_in_maps(x, Wq, bq, Wk, bk, Wv, bv):
    """Build per-core input maps (host-side shard + weight-folding/layout)."""
    import ml_dtypes

    bf = ml_dtypes.bfloat16
    f8 = ml_dtypes.float8_e4m3
    x = np.asarray(x, dtype=np.float32)
    Wq = np.asarray(Wq, dtype=np.float64)
    Wk = np.asarray(Wk, dtype=np.float64)
    bq = np.asarray(bq, dtype=np.float64)
    # MT[d',d] = (Wk^T Wq)[d',d] = M[d,d'] with M = Wq^T Wk (weight folding);
    # stored fp8 at x64 scale to clear the e4m3 subnormal threshold
    wmT = np.ascontiguousarray((64.0 * (Wk.T @ Wq)).astype(np.float32).astype(f8))
    wvT32 = np.asarray(Wv, dtype=np.float32).T
    wvT = np.ascontiguousarray(wvT32.astype(bf))
    wv8T = np.ascontiguousarray(wvT32.astype(f8))
    vcol = (SCALE * (Wk.T @ bq)).astype(np.float32).astype(bf)
    bv = np.asarray(bv, dtype=np.float32)
    kq = np.arange(QB)
    mask = np.where(kq[:, None] <= kq[None, :], 0.0, NEG).astype(np.float32)
    ones2 = np.ones(2, dtype=np.float32)
    in_maps = []
    for c in range(B):
        xT32 = np.ascontiguousarray(x[:, c, :].T)
        in_maps.append(
            {
                "xT": xT32.astype(bf), "xT8": xT32.astype(f8),
                "wmT": wmT, "wvT": wvT, "wv8T": wv8T, "vcol": vcol,
                "bv": bv, "mask": mask, "ones2": ones2,
            }
        )
    return in_maps


def get_nc(repeat=1):
    key = ("nc", repeat)
    if key not in _cache:
        _cache[key] = _build(repeat)
    return _cache[key]


def kernel(x, Wq, bq, Wk, bk, Wv, bv):
    from concourse.bass_utils import run_bass_kernel_spmd

    nc = get_nc()
    in_maps = _prep_in_maps(x, Wq, bq, Wk, bk, Wv, bv)
    res = run_bass_kernel_spmd(nc, in_maps, core_ids=list(range(B)))
    outs = np.stack([res.results[c]["out"] for c in range(B)], axis=0)  # (B, S, A)
    return np.ascontiguousarray(outs.transpose(1, 0, 2))  # (S, B, A)
